# revision 23
# baseline (speedup 1.0000x reference)
"""CommNet actor kernel for Trainium2 (Bass/Tile), 8-core data-parallel.

Math (per sample, A=32 agents, D=128 obs, H=64 hidden, NA=16 actions):
    h   = tanh(obs @ enc_w + enc_b)
    2 rounds of:  messages = h @ comm_w + comm_b
                  received = (sum_agents(messages) - messages) / (A-1)
                  h = tanh([h, received] @ upd_w + upd_b)
    out = tanh(h @ dec_w1 + dec_b1) @ dec_w2 + dec_b2

The round is folded on the host into  h' = tanh(h @ W1 + s @ W2 + b)  where
s = sum_agents(h), W1 = U_top - comm_w @ U_bot / (A-1), W2 = comm_w @ U_bot / (A-1),
b = comm_b @ U_bot + upd_b   (U_top/U_bot = upd_w[:H], upd_w[H:]).

Device layout: feature-major activations [feat, tok]. Each "unit" is 2048
tokens; the first 1024 tokens (T0) live on SBUF/PSUM partitions 0:64, the
second 1024 (T1) on partitions 64:128. All matmuls run in float32r (single-
pass PE mode; plain fp32 costs 2 half-speed passes). f32r only supports
tile_position (0,0), so the two halves are computed with block-diagonal
weights kron(I2, W) in one full-array matmul; the encoder stacks halves via
a zero-padded lhsT accumulation pair. tanh/reduce then process both halves
in single [128, 1024] instructions (full 128-lane utilization).

obs is pre-transposed on the host into the exact feature-major DMA layout, so
all HBM traffic is contiguous; the output is stored in DMA walk order and
transposed back on the host.
"""

import numpy as np
from contextlib import ExitStack

import concourse.bass as bass
import concourse.bacc as bacc
import concourse.tile as tile
from concourse import mybir
from concourse.bass_utils import run_bass_kernel_spmd

# Problem constants
B, A, D, H, NA = 16384, 32, 128, 64, 16
R = 2
NCORES = 8
S_CORE = B // NCORES          # 2048 samples per core
TOK = S_CORE * A              # 65536 tokens per core
HALF_TOK = 1024               # tokens per half-unit (32 samples)
UNIT_TOK = 2 * HALF_TOK       # 2048 tokens per unit
NU = TOK // UNIT_TOK          # 32 units per core
SAMP_HALF = HALF_TOK // A     # 32 samples per half-unit
CHUNK = 128                   # dec2 token chunk (output partition dim)
NCHUNK = HALF_TOK // CHUNK    # 8 chunks per half-unit
FP = mybir.dt.float32
FR = mybir.dt.float32r  # single-pass PE mode (fp32 is 2 half-speed passes)
BF = mybir.dt.bfloat16  # 1 cycle/col moving operand (f32r moves at 2 cyc/col)
TANH = mybir.ActivationFunctionType.Tanh


def _f(ap):
    return ap.bitcast(FP)


# wpack16 (bf16) column layout: encoder + both rounds
_C_ENC = 0              # enc_w                 [128, 64]   (T0 encoder)
_C_ENCP = 64            # [0 | enc_w]           [128, 128]  (T1 encoder, zero-pad)
_C_W1 = (192, 448)      # kron(I2, W1_r)        [128, 128] per round
_C_W2 = (320, 576)      # kron(I2, W2_r)        [128, 128] per round
_C_D1_16 = 704          # kron(I2, dec_w1)      [128, 128]
_C_D2_16 = 832          # kron(I2, dec_w2)      [128, 32]
NW16 = 864
# wpack (f32r) column layout: decoder + biases
_C_D1 = 0               # kron(I2, dec_w1)      [128, 128]
_C_D2 = 128             # kron(I2, dec_w2)      [128, 32]
_C_BE = 160             # bias cols: enc, r0, r1, dec1 (each stacked [b; b])
NW = 164


def build_body(ctx, tc, obs_t, wpack, out, n_units):
    nc = tc.nc
    wpool = ctx.enter_context(tc.tile_pool(name="w", bufs=1))
    obs_pool = ctx.enter_context(tc.tile_pool(name="obs", bufs=16))
    h_pool = ctx.enter_context(tc.tile_pool(name="h", bufs=24))
    s_pool = ctx.enter_context(tc.tile_pool(name="s", bufs=16))
    osb_pool = ctx.enter_context(tc.tile_pool(name="osb", bufs=12))
    ps_pool = ctx.enter_context(tc.tile_pool(name="ps", bufs=4, space="PSUM"))

    wpack, wpack16 = wpack
    w = wpool.tile([D, NW], FR)
    nc.sync.dma_start(out=w[:], in_=wpack)
    w16 = wpool.tile([D, NW16], BF)
    nc.sync.dma_start(out=w16[:], in_=wpack16)

    w_enc = w16[:, _C_ENC : _C_ENC + 64]
    w_encp = w16[:, _C_ENCP : _C_ENCP + 128]
    w1 = [w16[:, _C_W1[r] : _C_W1[r] + 128] for r in range(R)]
    w2 = [w16[:, _C_W2[r] : _C_W2[r] + 128] for r in range(R)]
    w_d1 = w[:, _C_D1 : _C_D1 + 128]
    w_d2 = w[:, _C_D2 : _C_D2 + 32]
    b_enc = _f(w[:, _C_BE : _C_BE + 1])
    b_r = [_f(w[:, _C_BE + 1 + r : _C_BE + 2 + r]) for r in range(R)]
    b_d1 = _f(w[:, _C_BE + 3 : _C_BE + 4])

    c0 = slice(0, 512)
    c1 = slice(512, 1024)
    lo = slice(0, 64)

    # out DRAM layout is the DMA walk order itself: [u, 32 rows, 1024 tok]
    # where row = half*16 + action (host transposes back afterwards)
    out_v = out.rearrange("(u r) t -> u r t", r=32)

    def emit_loads(u):
        obs0 = obs_pool.tile([D, HALF_TOK], BF, tag="obs")
        obs1 = obs_pool.tile([D, HALF_TOK], BF, tag="obs")
        nc.sync.dma_start(out=obs0[:], in_=obs_t[u, 0])
        nc.sync.dma_start(out=obs1[:], in_=obs_t[u, 1])
        return obs0, obs1

    def emit_enc_mms(obs0, obs1):
        ps_e = ps_pool.tile([128, HALF_TOK], FP, tag="ps")
        for cs in (c0, c1):
            nc.tensor.matmul(ps_e[:, cs], lhsT=w_encp, rhs=obs1[:, cs],
                             start=True, stop=False, skip_group_check=True)
        for cs in (c0, c1):
            nc.tensor.matmul(ps_e[lo, cs], lhsT=w_enc, rhs=obs0[:, cs],
                             start=False, stop=True, skip_group_check=True)
        return ps_e

    def emit_tanh(ps, bias, dt=FR):
        hh = h_pool.tile([128, HALF_TOK], dt, tag="h")
        nc.scalar.activation(hh[:], ps[:], TANH, bias=bias)
        return hh

    def emit_reduce(hh):
        s = s_pool.tile([128, SAMP_HALF], hh.dtype, tag="s")
        with nc.allow_low_precision(
            reason="float32r is 4-byte fp32; PE needs f32r-typed operands"
        ):
            nc.vector.reduce_sum(
                out=s[:],
                in_=hh.rearrange("p (g a) -> p g a", a=A),
                axis=mybir.AxisListType.X,
            )
        return s

    def emit_round_mms(r, hh, s):
        ns = SAMP_HALF // 2  # samples per 512-token column block
        ps_r = ps_pool.tile([128, HALF_TOK], FP, tag="ps")
        for cs in (c0, c1):
            nc.tensor.matmul(ps_r[:, cs], lhsT=w1[r], rhs=hh[:, cs],
                             start=True, stop=False, skip_group_check=True)
        for b, cs in ((0, c0), (1, c1)):
            sb = s[:, b * ns : (b + 1) * ns].unsqueeze(2).broadcast_to(
                [128, ns, A]
            )
            nc.tensor.matmul(ps_r[:, cs], lhsT=w2[r], rhs=sb,
                             start=False, stop=True, skip_group_check=True)
        return ps_r

    def emit_dec1_mms(hh):
        ps_d = ps_pool.tile([128, HALF_TOK], FP, tag="ps")
        for cs in (c0, c1):
            nc.tensor.matmul(ps_d[:, cs], lhsT=w_d1, rhs=hh[:, cs],
                             skip_group_check=True)
        return ps_d

    def emit_dec2_mms(pre):
        # dec2 feature-major: logits [32, 1024] in the top 32 partitions of a
        # shared-pool psum tile; rows 0:16 = T0 actions, 16:32 = T1
        po = ps_pool.tile([128, HALF_TOK], FP, tag="ps")
        for cs in (c0, c1):
            nc.tensor.matmul(po[0:32, cs], lhsT=w_d2, rhs=pre[:, cs],
                             skip_group_check=True)
        return po

    def emit_dec2_out(u, po):
        osb = osb_pool.tile([32, HALF_TOK], FP, tag="osb")
        nc.vector.tensor_copy(osb[:], po[0:32, :])
        nc.sync.dma_start(out=out_v[u], in_=osb[:])

    # Units are emitted in interleaved groups of three (software pipelining
    # at the emission level): one unit's matmuls fill the PE gaps left by
    # another unit's tanh/reduce stages. Without this the PE idles ~1.3us at
    # every stage boundary and the HAM clock-gate re-throttles it to 1.2 GHz.
    groups = []
    u0 = 0
    while u0 < n_units:
        g = 3 if n_units - u0 >= 3 else n_units - u0
        groups.append(list(range(u0, u0 + g)))
        u0 += g
    for grp in groups:
        obs = [emit_loads(u) for u in grp]
        ps = [emit_enc_mms(*o) for o in obs]
        hs = [emit_tanh(p, b_enc, BF) for p in ps]
        for r in range(R):
            ss = [emit_reduce(hh) for hh in hs]
            ps = [emit_round_mms(r, hh, s) for hh, s in zip(hs, ss)]
            dt_r = BF if r < R - 1 else FR
            hs = [emit_tanh(p, b_r[r], dt_r) for p in ps]
        ps = [emit_dec1_mms(hh) for hh in hs]
        pres = [emit_tanh(p, b_d1, FR) for p in ps]
        pos = [emit_dec2_mms(pre) for pre in pres]
        for u, po in zip(grp, pos):
            emit_dec2_out(u, po)

def _enable_ldw_opt():
    # Let walrus dedupe back-to-back LDWEIGHTS of the same stationary operand
    # (the default flags force it off).
    try:
        from concourse.compiler_utils import get_compiler_flags, set_compiler_flags

        flags = [
            f.replace("enable-ldw-opt=false", "enable-ldw-opt=true")
            for f in get_compiler_flags()
        ]
        set_compiler_flags(flags)
    except Exception:
        pass


def build_nc(n_units=NU):
    _enable_ldw_opt()
    nc = bacc.Bacc(None, target_bir_lowering=False, debug=False)
    obs_t = nc.declare_dram_parameter(
        "obs_t", [n_units, 2, D, HALF_TOK], BF, isOutput=False
    )
    wpack = nc.declare_dram_parameter("wpack", [D, NW], FR, isOutput=False)
    wpack16 = nc.declare_dram_parameter("wpack16", [D, NW16], BF, isOutput=False)
    out = nc.declare_dram_parameter(
        "out", [n_units * 32, HALF_TOK], FP, isOutput=True
    )
    with tile.TileContext(nc) as tc:
        with ExitStack() as ctx:
            build_body(ctx, tc, obs_t[:], (wpack[:], wpack16[:]), out[:], n_units)
    nc.compile()
    return nc


def fold_weights(enc_w, enc_b, comm_w, comm_b, upd_w, upd_b, dec_w1, dec_b1, dec_w2):
    """Host-side algebraic fold + packing into the wpack tensor (float64 math)."""
    import ml_dtypes

    f8 = np.float64
    denom = f8(max(A - 1, 1))
    wpack = np.zeros((D, NW), np.float32)
    wpack16 = np.zeros((D, NW16), np.float32)

    def bd(Wm):  # kron(I2, W) for [64, x] -> [128, 2x]
        Wm = np.asarray(Wm, np.float32)
        k, m = Wm.shape
        o = np.zeros((2 * k, 2 * m), np.float32)
        o[:k, :m] = Wm
        o[k:, m:] = Wm
        return o

    wpack16[:, _C_ENC : _C_ENC + 64] = np.asarray(enc_w, np.float32)
    wpack16[:, _C_ENCP + 64 : _C_ENCP + 128] = np.asarray(enc_w, np.float32)
    for r in range(R):
        C = np.asarray(comm_w[r], f8)
        Ut = np.asarray(upd_w[r][:H], f8)
        Ub = np.asarray(upd_w[r][H:], f8)
        G = C @ Ub / denom
        W1 = (Ut - G).astype(np.float32)
        W2 = G.astype(np.float32)
        br = (np.asarray(comm_b[r], f8) @ Ub + np.asarray(upd_b[r], f8)).astype(
            np.float32
        )
        wpack16[:, _C_W1[r] : _C_W1[r] + 128] = bd(W1)
        wpack16[:, _C_W2[r] : _C_W2[r] + 128] = bd(W2)
        wpack[0:64, _C_BE + 1 + r] = br
        wpack[64:128, _C_BE + 1 + r] = br
    wpack16[:, _C_D1_16 : _C_D1_16 + 128] = bd(dec_w1)
    wpack16[:, _C_D2_16 : _C_D2_16 + 32] = bd(dec_w2)
    wpack[:, _C_D1 : _C_D1 + 128] = bd(dec_w1)
    wpack[:, _C_D2 : _C_D2 + 32] = bd(dec_w2)
    be = np.asarray(enc_b, np.float32)
    wpack[0:64, _C_BE] = be
    wpack[64:128, _C_BE] = be
    bd1 = np.asarray(dec_b1, np.float32)
    wpack[0:64, _C_BE + 3] = bd1
    wpack[64:128, _C_BE + 3] = bd1
    return wpack, wpack16.astype(ml_dtypes.bfloat16)


def prep_obs(obs):
    """[B, A, D] -> [NCORES, NU, 2, D, HALF_TOK] feature-major bf16."""
    import ml_dtypes

    obs5 = np.asarray(obs, np.float32).reshape(NCORES, NU, 2, HALF_TOK, D)
    return np.ascontiguousarray(
        obs5.transpose(0, 1, 2, 4, 3).astype(ml_dtypes.bfloat16)
    )


_NC_CACHE = {}


def _get_nc(n_units=NU):
    if n_units not in _NC_CACHE:
        _NC_CACHE[n_units] = build_nc(n_units)
    return _NC_CACHE[n_units]


def kernel(
    obs,
    enc_w,
    enc_b,
    comm_w,
    comm_b,
    upd_w,
    upd_b,
    dec_w1,
    dec_b1,
    dec_w2,
    dec_b2,
    _trace=False,
    _trace_kwargs=None,
):
    wpack, wpack16 = fold_weights(
        enc_w, enc_b, comm_w, comm_b, upd_w, upd_b, dec_w1, dec_b1, dec_w2
    )
    obs_t = prep_obs(obs)
    nc = _get_nc()
    in_maps = [
        {"obs_t": obs_t[i], "wpack": wpack, "wpack16": wpack16}
        for i in range(NCORES)
    ]
    res = run_bass_kernel_spmd(
        nc,
        in_maps,
        core_ids=list(range(NCORES)),
        trace=_trace,
        **(_trace_kwargs or {}),
    )
    outs = np.stack([res.results[i]["out"] for i in range(NCORES)])
    # device order is [u, half*16+e, tok]; token t = u*2048 + half*1024 + tok
    outs = outs.reshape(NCORES, NU, 2, NA, HALF_TOK)
    outs = outs.transpose(0, 1, 2, 4, 3)  # -> [core, u, half, tok, e]
    logits = outs.reshape(B, A, NA) + np.asarray(dec_b2, np.float32)[None, None, :]
    if _trace:
        return logits.astype(np.float32), res
    return logits.astype(np.float32)
